# revision 2
# baseline (speedup 1.0000x reference)
"""Cross-attention kernel for 8 TRN2 NeuronCores.

Reference shapes: x [4, 2048, 1024], embeds [4, 2048, 1024],
Wq/Wk/Wv [1024, 1024] (+bias), Wo [1024, 1024] (+bias), H=16 heads, D=64.

Sharding: core c handles batch b = c//2 and head group hg = c%2 (8 heads,
attn-dim slice of 512).  Each core computes a partial output
outT_c [1024, 2048] = (ctx_c @ Wo[hg-slice]) ^T; the host sums the two
partials per batch (row-parallel Wo all-reduce done at unshard time) and
adds nothing else (bo is folded into the even core's partial).

Device dataflow per core (activations kept feature-major, "T" = [feat, tok]):
  QT = Wq_c^T @ xT      [512, 2048]   (fp32r matmuls, psum fp32)
  KT = Wk_c^T @ embT    [512, 2048]
  V  = embT^T-proj      [2048, 512]   token-major, + ones column per head
  per head h, lq-half: ST = K_h @ Q_h^T  -> exp (ACT, scale=1/8) -> E
                       [C';denom] = [V_h|1]^T @ E   (ones-column trick)
                       CT_h = C' * (1/denom)  (recip + partition_broadcast)
  outT = Wo_c^T @ CT    [1024, 2048]  + bo (even core only)
Softmax skips the max-subtraction: scores ~ N(0,1), |s| < ~7, exp is safe
in fp32 and matches the reference softmax mathematically.
"""

import sys

if "/opt/trn_rl_repo" not in sys.path:
    sys.path.insert(0, "/opt/trn_rl_repo")

import numpy as np

import concourse.bass as bass  # noqa: F401  (engine namespaces live on nc)
import concourse.mybir as mybir
import concourse.tile as tile
from concourse import bacc
from concourse.bass_utils import run_bass_kernel_spmd

P = 128
B, LQ, LK, DIM = 4, 2048, 2048, 1024
H, D = 16, 64
ADC = 512          # per-core attention dim (8 heads x 64)
NHC = 8            # heads per core
SCALE = 1.0 / 8.0
F32 = mybir.dt.float32
FR = mybir.dt.float32r
EXP = mybir.ActivationFunctionType.Exp

K_T = DIM // P     # 8 contraction tiles for projections
M_AD = ADC // P    # 4 ad partition tiles
T_LK = LK // P     # 16 lk tiles
VW = NHC * (D + 1)  # 520: V block width per lk tile (64 cols + ones col per head)

_CACHE = {}


def _build():
    nc = bacc.Bacc("TRN2", target_bir_lowering=False, debug=False)

    xT = nc.dram_tensor("xT", [DIM, LQ], FR, kind="ExternalInput").ap()
    embT = nc.dram_tensor("embT", [DIM, LK], FR, kind="ExternalInput").ap()
    Wq = nc.dram_tensor("Wq", [DIM, ADC], FR, kind="ExternalInput").ap()
    Wk = nc.dram_tensor("Wk", [DIM, ADC], FR, kind="ExternalInput").ap()
    Wv = nc.dram_tensor("Wv", [DIM, ADC], FR, kind="ExternalInput").ap()
    Wo = nc.dram_tensor("Wo", [ADC, DIM], FR, kind="ExternalInput").ap()
    bq = nc.dram_tensor("bq", [P, M_AD], F32, kind="ExternalInput").ap()
    bk = nc.dram_tensor("bk", [P, M_AD], F32, kind="ExternalInput").ap()
    bvb = nc.dram_tensor("bvb", [P, ADC], F32, kind="ExternalInput").ap()
    bo = nc.dram_tensor("bo", [P, DIM // P], F32, kind="ExternalInput").ap()
    outT = nc.dram_tensor("outT", [DIM, LQ], F32, kind="ExternalOutput").ap()

    with tile.TileContext(nc) as tc:
        with tc.tile_pool(name="resident", bufs=1) as res:
            QT = [res.tile([P, LQ], FR, name=f"qt{m}") for m in range(M_AD)]
            KT = [res.tile([P, LK], FR, name=f"kt{m}") for m in range(M_AD)]
            V = res.tile([P, T_LK * VW], FR, name="v")
            CT = [res.tile([P, LQ], FR, name=f"ct{p}") for p in range(M_AD)]
            WO = res.tile([P, ADC // P, DIM], FR, name="wo")
            bq_sb = res.tile([P, M_AD], F32, name="bq")
            bk_sb = res.tile([P, M_AD], F32, name="bk")
            bvb_sb = res.tile([P, ADC], F32, name="bvb")
            bo_sb = res.tile([P, DIM // P], F32, name="bo")

            nc.sync.dma_start(WO[:], Wo.rearrange("(k p) n -> p k n", p=P))
            nc.sync.dma_start(bq_sb[:], bq[:])
            nc.sync.dma_start(bk_sb[:], bk[:])
            nc.sync.dma_start(bvb_sb[:], bvb[:])
            nc.sync.dma_start(bo_sb[:], bo[:])

            # ones columns (col 64 of each head's 65-wide block) for the
            # fused-denominator C matmul.  memset can't write fp32r, so
            # synthesize 1.0 on DVE as in0*0 + 1.
            zsrc = res.tile([P, NHC], F32, name="zsrc")
            nc.gpsimd.memset(zsrc[:], 0.0)
            for t in range(T_LK):
                blk = V[:, t * VW:(t + 1) * VW].rearrange(
                    "p (a b) -> p a b", b=D + 1)
                nc.vector.tensor_scalar(
                    blk[:, :, D:D + 1],
                    zsrc[:].rearrange("p (a b) -> p a b", b=1),
                    0.0, 1.0,
                    op0=mybir.AluOpType.mult, op1=mybir.AluOpType.add)

            # ---------------- projections ----------------
            with tc.tile_pool(name="wproj", bufs=2) as wpool, \
                 tc.tile_pool(name="stream", bufs=4) as spool, \
                 tc.tile_pool(name="pjp", bufs=1, space="PSUM") as pjp, \
                 tc.tile_pool(name="pjv", bufs=2, space="PSUM") as pjv:

                for (w_dram, b_sb, out_tiles, src) in (
                        (Wq, bq_sb, QT, xT), (Wk, bk_sb, KT, embT)):
                    w_sb = wpool.tile([P, K_T, ADC], FR, name="w")
                    nc.sync.dma_start(
                        w_sb[:], w_dram.rearrange("(k p) m -> p k m", p=P))
                    for n in range(LQ // 512):
                        pps = [pjp.tile([P, 512], F32, name=f"pp{m}")
                               for m in range(M_AD)]
                        for k in range(K_T):
                            xt = spool.tile([P, 512], FR, name="xs")
                            nc.sync.dma_start(
                                xt[:],
                                src[k * P:(k + 1) * P, n * 512:(n + 1) * 512])
                            for m in range(M_AD):
                                nc.tensor.matmul(
                                    pps[m][:],
                                    w_sb[:, k, m * P:(m + 1) * P],
                                    xt[:],
                                    start=(k == 0), stop=(k == K_T - 1))
                        for m in range(M_AD):
                            nc.vector.tensor_scalar_add(
                                out_tiles[m][:, n * 512:(n + 1) * 512],
                                pps[m][:], b_sb[:, m:m + 1])

                wv_sb = wpool.tile([P, K_T, ADC], FR, name="w")
                nc.sync.dma_start(
                    wv_sb[:], Wv.rearrange("(k p) m -> p k m", p=P))
                for t in range(T_LK):
                    psv = pjv.tile([P, ADC], F32, name="pv")
                    for k in range(K_T):
                        et = spool.tile([P, P], FR, name="es")
                        nc.sync.dma_start(
                            et[:], embT[k * P:(k + 1) * P, t * P:(t + 1) * P])
                        nc.tensor.matmul(psv[:], et[:], wv_sb[:, k, :],
                                         start=(k == 0), stop=(k == K_T - 1))
                    vdst = V[:, t * VW:(t + 1) * VW].rearrange(
                        "p (a b) -> p a b", b=D + 1)[:, :, 0:D]
                    nc.vector.tensor_tensor(
                        vdst,
                        psv[:].rearrange("p (a b) -> p a b", b=D),
                        bvb_sb[:].rearrange("p (a b) -> p a b", b=D),
                        op=mybir.AluOpType.add)

            # ---------------- attention ----------------
            with tc.tile_pool(name="aps", bufs=2, space="PSUM") as aps, \
                 tc.tile_pool(name="apc", bufs=2, space="PSUM") as apc, \
                 tc.tile_pool(name="etp", bufs=3) as etp, \
                 tc.tile_pool(name="small", bufs=2) as small:
                for h in range(NHC):
                    mt, ro = h // 2, (h % 2) * D
                    vcol = h * (D + 1)
                    for half in range(2):
                        q0 = half * 1024
                        pc = apc.tile([D + 1, 1024], F32, name="pc")
                        for t in range(T_LK):
                            ps = aps.tile([P, 1024], F32, name="ps")
                            for nn in range(2):
                                nc.tensor.matmul(
                                    ps[:, nn * 512:(nn + 1) * 512],
                                    KT[mt][ro:ro + D, t * P:(t + 1) * P],
                                    QT[mt][ro:ro + D,
                                           q0 + nn * 512:q0 + (nn + 1) * 512],
                                    start=True, stop=True)
                            et = etp.tile([P, 1024], FR, name="et")
                            nc.scalar.activation(et[:], ps[:], EXP, scale=SCALE)
                            for nn in range(2):
                                nc.tensor.matmul(
                                    pc[:, nn * 512:(nn + 1) * 512],
                                    V[:, t * VW + vcol:t * VW + vcol + D + 1],
                                    et[:, nn * 512:(nn + 1) * 512],
                                    start=(t == 0), stop=(t == T_LK - 1))
                        r1 = small.tile([1, 1024], F32, name="r1")
                        nc.vector.reciprocal(r1[:], pc[D:D + 1, :])
                        rb = small.tile([D, 1024], F32, name="rb")
                        nc.gpsimd.partition_broadcast(rb[:], r1[0:1, :])
                        nc.vector.tensor_tensor(
                            CT[mt][ro:ro + D, q0:q0 + 1024],
                            pc[0:D, :], rb[:], op=mybir.AluOpType.mult)

            # ---------------- output projection ----------------
            with tc.tile_pool(name="ops", bufs=4, space="PSUM") as ops, \
                 tc.tile_pool(name="ostage", bufs=4) as ostage:
                for m in range(DIM // P):
                    for n in range(LQ // 512):
                        po = ops.tile([P, 512], F32, name="po")
                        for kk in range(ADC // P):
                            nc.tensor.matmul(
                                po[:],
                                WO[:, kk, m * P:(m + 1) * P],
                                CT[kk][:, n * 512:(n + 1) * 512],
                                start=(kk == 0), stop=(kk == ADC // P - 1))
                        ot = ostage.tile([P, 512], F32, name="ot")
                        nc.vector.tensor_scalar_add(ot[:], po[:],
                                                    bo_sb[:, m:m + 1])
                        nc.sync.dma_start(
                            outT[m * P:(m + 1) * P, n * 512:(n + 1) * 512],
                            ot[:])

    nc.compile()
    return nc


def _in_maps(x, embeds, Wq, bq, Wk, bk, Wv, bv, Wo, bo):
    f = np.float32
    maps = []
    for c in range(8):
        b, hg = c // 2, c % 2
        s = slice(hg * ADC, (hg + 1) * ADC)
        bo_c = bo if hg == 0 else np.zeros_like(bo)
        maps.append({
            "xT": np.ascontiguousarray(x[b].T, dtype=f),
            "embT": np.ascontiguousarray(embeds[b].T, dtype=f),
            "Wq": np.ascontiguousarray(Wq[:, s], dtype=f),
            "Wk": np.ascontiguousarray(Wk[:, s], dtype=f),
            "Wv": np.ascontiguousarray(Wv[:, s], dtype=f),
            "Wo": np.ascontiguousarray(Wo[s, :], dtype=f),
            "bq": np.ascontiguousarray(
                bq[s].reshape(M_AD, P).T, dtype=f),
            "bk": np.ascontiguousarray(
                bk[s].reshape(M_AD, P).T, dtype=f),
            "bvb": np.ascontiguousarray(
                np.tile(bv[s], (P, 1)), dtype=f),
            "bo": np.ascontiguousarray(
                bo_c.reshape(DIM // P, P).T, dtype=f),
        })
    return maps


def kernel(x, embeds, Wq, bq, Wk, bk, Wv, bv, Wo, bo, _trace=False,
           _tmpdir=None):
    x = np.asarray(x); embeds = np.asarray(embeds)
    Wq = np.asarray(Wq); bq = np.asarray(bq)
    Wk = np.asarray(Wk); bk = np.asarray(bk)
    Wv = np.asarray(Wv); bv = np.asarray(bv)
    Wo = np.asarray(Wo); bo = np.asarray(bo)

    if "nc" not in _CACHE:
        _CACHE["nc"] = _build()
    nc = _CACHE["nc"]

    maps = _in_maps(x, embeds, Wq, bq, Wk, bk, Wv, bv, Wo, bo)
    res = run_bass_kernel_spmd(nc, maps, core_ids=list(range(8)),
                               trace=_trace, tmpdir=_tmpdir)
    if _trace:
        _CACHE["last_exec_time_ns"] = res.exec_time_ns
        _CACHE["last_results"] = res

    out = np.empty((B, LQ, DIM), np.float32)
    for b in range(B):
        acc = res.results[2 * b]["outT"] + res.results[2 * b + 1]["outT"]
        out[b] = acc.T
    return out
